# revision 4
# baseline (speedup 1.0000x reference)
"""Trainium2 Bass kernel for nn_CompressiveMemory_57750130262084.

The reference computes (B=8, S=4096, DK=DV=1024):
    sigma  = elu(query) + 1                                  [B,S,DK]
    memory = einsum('bkd,bsv->bkv', swap(sigma), value)      [B,DK,DV]
    z_norm = sum_s sigma                                     [B,DK]
    out    = einsum('bsd,bkv->bsv', sigma, memory)
           / einsum('bsd,bk->bs',  sigma, z_norm)[..., None]

Every einsum uses disjoint summed subscripts, so each factorises into
outer products of independent reductions; algebra collapses to
    out[b,s,v] = sum_s value[b,s,v]        (exactly; query cancels)

So the kernel is a column-sum of `value` over S, broadcast over S.
Sharding: data-parallel over batch, one NeuronCore per batch element.
Per-core work: read 16 MB, reduce 4096 rows -> 1 row, write 16 MB.

v2 schedule (from trace analysis of v1 @ 120.3us):
  * SDMA engine 15 is ~12% slower than engines 0-14 (181ns vs 161ns
    per 4KB packet, both directions). With a uniform row->partition
    mapping it owns 1/16 of the bytes and straggles ~6-8us at the end
    of BOTH the read and the write stream. Row assignment to
    partitions is free here (we sum everything / all output rows are
    identical), so partitions served by engine 15 ({92-95,124-127})
    get 28 DRAM rows, other partitions 32-33 ("block" mapping:
    partition p owns a contiguous row range). Expected per-engine
    busy ~41.9us, balanced.
  * No per-chunk PE matmuls (fp32 matmul is 4 passes/chunk and
    backlogged v1's tail by ~13us). DVE folds each input window's
    rows into a per-window partial (tensor_add chain, 1.23us/chunk);
    the PE reduces each finished partial across partitions into
    accumulating PSUM while later windows stream in.
  * The last three windows (2+2+1 row-slots) are converted to bf16
    (ACT) so their PE passes are single-pass; post-stream tail is
    fold(1.23)+convert+2 short matmuls+PSUM copy ~4.5us instead of
    ~17us. bf16 on ~15% of the data adds ~7e-4 relative error
    (harness gate is 2e-2).
  * Output written as broadcast-source DMAs with the same skewed
    row mapping; last write DMA is small so its completion receipt
    isn't behind 4MB of data.
"""

import numpy as np

B, S, D = 8, 4096, 1024
P = 128
H = 512                 # PSUM bank width in f32 (matmul N limit)

# Block row->partition mapping, skewed away from slow SDMA engine 15
# (partitions 92-95 and 124-127): (p0, p1, rows_per_partition, row0).
SEGMENTS = [
    (0, 32, 33, 0),
    (32, 92, 32, 1056),
    (92, 96, 28, 2976),
    (96, 124, 32, 3088),
    (124, 128, 28, 3984),
]
assert SEGMENTS[-1][3] + (SEGMENTS[-1][1] - SEGMENTS[-1][0]) * SEGMENTS[-1][2] == S

# Input windows over per-partition row-slots [a, b). A segment with R
# rows participates in slots [0, R). Windows past slot 28 are ragged.
IN_WINDOWS = [(0, 5), (5, 10), (10, 15), (15, 20), (20, 25), (25, 28),
              (28, 30), (30, 32), (32, 33)]
BF16_WINDOWS = {6, 7, 8}          # tail windows take the 1-pass bf16 PE path
OUT_WINDOWS = [(0, 14), (14, 28), (28, 32), (32, 33)]

_CACHE: dict = {}


def _build_program():
    import concourse.mybir as mybir
    import concourse.tile as tile
    from concourse import bacc

    f32 = mybir.dt.float32
    bf16 = mybir.dt.bfloat16
    nc = bacc.Bacc("TRN2", target_bir_lowering=False, debug=False,
                   num_devices=B, enable_asserts=False)
    v = nc.declare_dram_parameter("value", [S, D], f32, isOutput=False)
    o = nc.declare_dram_parameter("out", [S, D], f32, isOutput=True)

    def seg_rows(p0, p1, R, c0, a, b):
        """DRAM AP for row-slots [a,b) of partitions [p0,p1) (block map)."""
        blk = v[c0 : c0 + (p1 - p0) * R].rearrange("(p r) m -> p r m", r=R)
        return blk[:, a:b]

    def seg_rows_out(p0, p1, R, c0, a, b):
        blk = o[c0 : c0 + (p1 - p0) * R].rearrange("(p r) m -> p r m", r=R)
        return blk[:, a:b]

    with tile.TileContext(nc) as tc:
        with (
            tc.tile_pool(name="in", bufs=1) as in_pool,
            tc.tile_pool(name="part", bufs=1) as part_pool,
            tc.tile_pool(name="ones", bufs=1) as ones_pool,
            tc.tile_pool(name="bcast", bufs=1) as bcast_pool,
            tc.tile_pool(name="psum", bufs=1, space="PSUM") as psum_pool,
        ):
            ones_f = ones_pool.tile([P, P], f32, tag="ones_f")
            nc.vector.memset(ones_f[:], 1.0)
            ones_b = ones_pool.tile([P, P], bf16, tag="ones_b")
            nc.vector.memset(ones_b[:], 1.0)

            ps = psum_pool.tile([P, D], f32)

            # Window tiles (distinct buffers; ragged ones need their
            # no-DMA partition regions zeroed before the folds read them).
            wtiles = []
            for wi, (a, b) in enumerate(IN_WINDOWS):
                n = b - a
                t = in_pool.tile([P, n * D], f32, tag=f"w{wi}")
                wtiles.append(t)
                # Zero regions no DMA writes (compute ops must start at a
                # multiple-of-32 partition, so zero an aligned superset).
                if a >= 32:
                    nc.vector.memset(t[32:64, :], 0.0)
                    nc.vector.memset(t[64:128, :], 0.0)
                elif a >= 28:
                    nc.vector.memset(t[64:128, :], 0.0)

            # All input DMAs, in window order.
            for wi, (a, b) in enumerate(IN_WINDOWS):
                t = wtiles[wi]
                for (p0, p1, R, c0) in SEGMENTS:
                    b_eff = min(b, R)
                    if b_eff <= a:
                        continue
                    dst = t[p0:p1].rearrange("p (n m) -> p n m", n=b - a)
                    nc.sync.dma_start(dst[:, 0 : b_eff - a], seg_rows(p0, p1, R, c0, a, b_eff))

            # Fold + per-window partial reduction into PSUM.
            n_pe = 0
            total_pe = len(IN_WINDOWS)
            for wi, (a, b) in enumerate(IN_WINDOWS):
                t = wtiles[wi]
                n = b - a
                if n == 1:
                    partial = t
                else:
                    partial = part_pool.tile([P, D], f32, tag=f"p{wi % 3}")
                    nc.vector.tensor_add(partial[:], t[:, 0:D], t[:, D : 2 * D])
                    for i in range(2, n):
                        nc.vector.tensor_add(partial[:], partial[:], t[:, i * D : (i + 1) * D])
                if wi in BF16_WINDOWS:
                    pb = part_pool.tile([P, D], bf16, tag=f"pb{wi % 3}")
                    nc.scalar.copy(pb[:], partial[:])
                    mm_in, mm_ones = pb, ones_b
                else:
                    mm_in, mm_ones = partial, ones_f
                for h in range(2):
                    nc.tensor.matmul(
                        ps[:, h * H : (h + 1) * H],
                        mm_ones[:],
                        mm_in[:, h * H : (h + 1) * H],
                        start=(n_pe == 0),
                        stop=(n_pe == total_pe - 1),
                    )
                n_pe += 1

            # PSUM -> SBUF in parallel halves (DVE + ACT).
            bc = bcast_pool.tile([P, D], f32)
            nc.vector.tensor_copy(bc[:, 0:H], ps[:, 0:H])
            nc.scalar.copy(bc[:, H:D], ps[:, H:D])

            # Output: broadcast bc rows to all DRAM rows, same skewed map.
            for (a, b) in OUT_WINDOWS:
                for (p0, p1, R, c0) in SEGMENTS:
                    b_eff = min(b, R)
                    if b_eff <= a:
                        continue
                    src = bc[p0:p1].unsqueeze(1).to_broadcast((p1 - p0, b_eff - a, D))
                    nc.sync.dma_start(seg_rows_out(p0, p1, R, c0, a, b_eff), src)

    nc.compile()
    return nc


def _get_program():
    if "nc" not in _CACHE:
        _CACHE["nc"] = _build_program()
    return _CACHE["nc"]


def kernel(query: np.ndarray, value: np.ndarray) -> np.ndarray:
    from concourse.bass_utils import run_bass_kernel_spmd

    del query  # output is exactly independent of query (see module docstring)
    value = np.ascontiguousarray(value, dtype=np.float32)
    assert value.shape == (B, S, D)

    nc = _get_program()
    in_maps = [{"value": value[b]} for b in range(B)]
    try:
        res = run_bass_kernel_spmd(nc, in_maps, list(range(B)))
    except Exception:
        # The tunneled runtime occasionally surfaces a transient
        # NRT_EXEC_UNIT_UNRECOVERABLE on the first dispatch; retry once.
        import time

        time.sleep(2.0)
        res = run_bass_kernel_spmd(nc, in_maps, list(range(B)))
    return np.stack([res.results[b]["out"] for b in range(B)], axis=0)
